# revision 1
# baseline (speedup 1.0000x reference)
"""Trainium2 Bass kernel for CantorMultiheadFusion.

Reference math:
    h      = x @ W_in^T                        # [B,S,D]
    d[s,k] = distances[s, routes[s,k]]
    w      = softmax(-d, axis=-1)              # [S,K]
    fused  = sum_k w[s,k] * h[:, routes[s,k]]  # [B,S,D]  (head reshape is a no-op)
    out    = fused @ W_out^T + b_out + x

Because the fusion weights are shared across the feature dim, the gather
commutes with both projections:
    out = (A @ x) @ (W_out @ W_in)^T + b_out + x
where A[s,j] = C[s,j] * exp(-distances[s,j]) / denom(s),
      C[s,j] = #{k : routes[s,k] == j}   (integer multiplicity),
      denom(s) = sum_j C[s,j] * exp(-distances[s,j]).
Duplicated route entries share the same distance, so the count matrix C is
exact. On device this is computed as exp(ln(C) - D) with ln(C) built
host-side from the int32 routes tensor alone (index marshalling; C=0 maps
to -1e4 so exp underflows to +0). All float math on the actual inputs
(exp, normalization, matmuls, residual) runs on device.

Sharding: sequence-parallel over S across 8 cores (256 rows each). x is
replicated since the A@x contraction needs all S rows. Layouts keep the
contraction dim on partitions with no on-chip transposes:
  stepA: t^T[e,s] += x[b,j-tile](lhsT) @ numerT[j-tile](rhs)   (j contract)
  stepB: out[s,i] += tT[s-chunk](lhsT) @ WcT(rhs)              (e contract)
stepA packs two accumulation groups per PSUM bank using a single
start=True per bank (start clears the whole bank's has_written bits, so
the second group relies on cleared bits to overwrite on first write) —
this lets both batches accumulate concurrently in 4 banks.

Precision plan (host casts are pure dtype marshalling): the matmul
datapath (x, softmax numerator, distances, ln(C), weights) streams in
bf16; the residual path and output stay fp32. Measured rel err ~5e-4.
Set STEPA_FP8=True to switch the x/numer stream to fp8e4m3 (~3us faster,
rel err ~5e-3).
"""

import os
import sys

import numpy as np

for _p in ("/opt/trn_rl_repo",):
    if os.path.isdir(_p) and _p not in sys.path:
        sys.path.insert(0, _p)

# Some container snapshots lack antenv.axon_hooks (the axon NTFF profile
# hook); stub it so run_bass_kernel_spmd(trace=True) degrades gracefully.
def _ensure_axon_hooks_stub():
    import types
    try:
        import antenv.axon_hooks  # noqa: F401
    except ModuleNotFoundError:
        try:
            import antenv
        except ModuleNotFoundError:
            return
        _stub = types.ModuleType("antenv.axon_hooks")
        _stub.get_axon_ntff_profile_hook = lambda: None
        sys.modules["antenv.axon_hooks"] = _stub
        antenv.axon_hooks = _stub


_ensure_axon_hooks_stub()

B, S, D, K = 2, 2048, 512, 64
N_CORES = 8
SLOC = S // N_CORES          # 256 sequence rows per core
NJ = S // 128                # 16 contraction tiles
NE = D // 128                # 4 feature chunks
NSC = SLOC // 128            # 2 seq chunks per core
JG = 4                       # j-tiles per streamed DMA group
STEPA_FP8 = False             # fp8e4m3 vs bf16 for the x/numer stream
NJG = NJ // JG               # stream groups

_CACHE = {}
LAST_RESULTS = None


def _build_nc(with_bias=True):
    import concourse.bacc as bacc
    import concourse.mybir as mybir
    import concourse.tile as tile

    F32 = mybir.dt.float32
    BF16 = mybir.dt.bfloat16
    F8 = mybir.dt.float8e4 if STEPA_FP8 else mybir.dt.bfloat16
    MUL = mybir.AluOpType.mult
    ADD = mybir.AluOpType.add

    nc = bacc.Bacc("TRN2", target_bir_lowering=False, debug=False, num_devices=1)

    x_d = nc.dram_tensor("x", [B, S, D], F8, kind="ExternalInput").ap()
    distT_d = nc.dram_tensor("distT", [S, SLOC], BF16, kind="ExternalInput").ap()
    lnct_d = nc.dram_tensor("lnct", [S, SLOC], BF16, kind="ExternalInput").ap()
    xres_d = nc.dram_tensor("xres", [B, SLOC, D], F32, kind="ExternalInput").ap()
    win_d = nc.dram_tensor("w_in", [D, D], BF16, kind="ExternalInput").ap()
    woutT_d = nc.dram_tensor("w_outT", [D, D], BF16, kind="ExternalInput").ap()
    bout_d = nc.dram_tensor("b_out", [1, D], F32, kind="ExternalInput").ap()
    out_d = nc.dram_tensor("out", [B, SLOC, D], F32, kind="ExternalOutput").ap()

    with tile.TileContext(nc) as tc:
        with (
            tc.tile_pool(name="big", bufs=1) as big,
            tc.tile_pool(name="dstream", bufs=2) as dstream,
            tc.tile_pool(name="cstream", bufs=2) as cstream,
            tc.tile_pool(name="sstream", bufs=2) as sstream,
            tc.tile_pool(name="pa", bufs=4, space="PSUM") as pa,
            tc.tile_pool(name="pden", bufs=1, space="PSUM") as pdenp,
            tc.tile_pool(name="ptr", bufs=2, space="PSUM") as ptr,
        ):
            # ---- persistent SBUF ----
            xbuf = big.tile([128, B * NJ * D], F8)        # (b,j) -> [128j, 512e]
            numer = big.tile([128, NJ * SLOC], F8)        # (j)   -> [128j, 256s]
            win_sb = big.tile([128, 4 * D], BF16)         # (a)   -> [128a, 512e]
            woutT_sb = big.tile([128, 4 * D], BF16)       # (a)   -> [128a, 512i]
            wcT = big.tile([128, 4 * D], BF16)            # (e)   -> [128e, 512i]
            bias_sb = big.tile([1, D], F32)
            bias_bc = big.tile([128, D], F32)
            ones_r = big.tile([1, 128], F32)
            onescol_8 = big.tile([128, 1], F8)
            xres_sb = big.tile([128, 2 * B * D], F32)     # (b,sc) -> [128s, 512e]
            resb = big.tile([128, 2 * B * D], F32)        # xres + bias
            tT = big.tile([128, B * NE * SLOC], BF16)     # (b,ec) -> [128e, 256s]
            outbuf = big.tile([128, 2 * B * D], F32)      # (b,sc) -> [128s, 512i]
            rdT = big.tile([128, NSC], F32)

            def xsl(b, j, n=1):
                o = (b * NJ + j) * D
                return xbuf[:, o:o + n * D]

            def nsl(j, n=1):
                return numer[:, j * SLOC:(j + n) * SLOC]

            def tsl(buf, b, c):
                o = (b * NE + c) * SLOC
                return buf[:, o:o + SLOC]

            def bsl(buf, b, sc):
                o = (b * NSC + sc) * D
                return buf[:, o:o + D]

            nc.vector.memset(onescol_8[:], 1.0)
            nc.vector.memset(ones_r[:], 1.0)

            # stepA PSUM: bank (b,p) holds ec=2p (cols :SLOC) and ec=2p+1
            # (cols SLOC:). Exactly one start=True per bank (j==0, even ec).
            pdT = pdenp.tile([128, NSC], F32)
            pts = {(b, p): pa.tile([128, 2 * SLOC], F32,
                                   name=f"pts{b}_{p}", tag="acc")
                   for b in range(B) for p in range(NE // 2)}

            def pta(b, ec):
                return pts[(b, ec // 2)][:, (ec % 2) * SLOC:(ec % 2 + 1) * SLOC]

            # ---- streamed inputs: ONE queue (SP) so the serial DMA device
            # serves in exactly this order: bias, d0,c0,x00,x10, W,W,
            # d1,c1,x01,x11, d2,... The weights ride in the first gap so
            # WcT fills PE idle time after stepA group 0. ----
            if with_bias:
                nc.sync.dma_start(out=bias_sb[:1, :], in_=bout_d[:, :])
                pb = ptr.tile([128, D], F32, name="pb", tag="tr")
                nc.tensor.matmul(pb[:], lhsT=ones_r[:1, :], rhs=bias_sb[:1, :],
                                 start=True, stop=True)
                nc.vector.tensor_copy(bias_bc[:], pb[:])
            else:
                # b_out is all-zero for this invocation (host-dispatched
                # program variant): the bias DMA/broadcast and residual
                # pre-adds are skipped; the epilogue adds xres directly.
                nc.vector.memset(bias_sb[:1, :], 0.0)

            # Variable-size stream groups: small first groups let stepA
            # start sooner; the final group de-interleaves the batches so
            # b=0's PSUM stop + tT copies + reciprocal hide under b=1's MMs.
            GROUPS = [(0, 1), (1, 2), (3, 3), (6, 3), (9, 2), (11, 2), (13, 3)]
            LASTG = len(GROUPS) - 1

            def stepa_j(j, b):
                if b == 0:
                    for sc in range(NSC):
                        nc.tensor.matmul(
                            pdT[:, sc:sc + 1],
                            lhsT=nsl(j)[:, sc * 128:(sc + 1) * 128],
                            rhs=onescol_8[:, :1],
                            start=(j == 0 and sc == 0),
                            stop=(j == NJ - 1 and sc == NSC - 1),
                            skip_group_check=True)
                for ec in range(NE):
                    nc.tensor.matmul(
                        pta(b, ec),
                        lhsT=xsl(b, j)[:, ec * 128:(ec + 1) * 128],
                        rhs=nsl(j),
                        start=(j == 0 and ec % 2 == 0),
                        stop=(j == NJ - 1 and ec % 2 == 1),
                        skip_group_check=True)

            def tt_copies(b, eng):
                for p in range(NE // 2):
                    dst = tT[:, (b * NE + 2 * p) * SLOC:
                             (b * NE + 2 * p + 2) * SLOC]
                    if eng == "dve":
                        nc.vector.tensor_copy(dst, pts[(b, p)][:])
                    else:
                        nc.scalar.copy(dst, pts[(b, p)][:])

            for g, (j0, jn) in enumerate(GROUPS):
                dt_t = dstream.tile([128, jn * SLOC], BF16, name=f"dt{g}", tag="dt")
                nc.sync.dma_start(
                    out=dt_t[:].rearrange("p (j s) -> p j s", j=jn),
                    in_=distT_d[j0 * 128:(j0 + jn) * 128, :]
                        .rearrange("(j p) s -> p j s", p=128))
                ct_t = cstream.tile([128, jn * SLOC], BF16, name=f"ct{g}", tag="ct")
                nc.sync.dma_start(
                    out=ct_t[:].rearrange("p (j s) -> p j s", j=jn),
                    in_=lnct_d[j0 * 128:(j0 + jn) * 128, :]
                        .rearrange("(j p) s -> p j s", p=128))
                for b in range(B):
                    nc.sync.dma_start(
                        out=xsl(b, j0, jn).rearrange("p (j e) -> p j e", j=jn),
                        in_=x_d[b, j0 * 128:(j0 + jn) * 128, :]
                            .rearrange("(j p) e -> p j e", p=128))
                if g == 0:
                    nc.sync.dma_start(
                        out=win_sb[:].rearrange("p (t e) -> p t e", t=4),
                        in_=win_d.rearrange("(t p) e -> p t e", p=128))
                    nc.sync.dma_start(
                        out=woutT_sb[:].rearrange("p (t i) -> p t i", t=4),
                        in_=woutT_d.rearrange("(t p) i -> p t i", p=128))
                # numer: per-j-tile sub (DVE) + exp (ACT) so each stepA
                # matmul gates on its own tile, not the whole group
                for jj in range(jn):
                    sb_t = sstream.tile([128, SLOC], BF16,
                                        name=f"sb{g}_{jj}", tag="sb")
                    nc.vector.tensor_sub(sb_t[:],
                                         ct_t[:, jj * SLOC:(jj + 1) * SLOC],
                                         dt_t[:, jj * SLOC:(jj + 1) * SLOC])
                    nc.scalar.activation(nsl(j0 + jj), sb_t[:],
                                         mybir.ActivationFunctionType.Exp)
                # stepA for this group's j-tiles
                if g < LASTG:
                    for j in range(j0, j0 + jn):
                        stepa_j(j, 0)
                        stepa_j(j, 1)
                else:
                    for j in range(j0, j0 + jn):
                        stepa_j(j, 0)
                    tt_copies(0, "dve")
                    nc.vector.reciprocal(rdT[:], pdT[:])
                    for j in range(j0, j0 + jn):
                        stepa_j(j, 1)
                if g == 0:
                    # WcT[e,i] = sum_a W_in[a,e] * W_outT[a,i] — after the
                    # first stepA group so PE starts on stream data ASAP
                    for ec in range(4):
                        pw = ptr.tile([128, D], F32, name=f"pw{ec}", tag="tr")
                        for at in range(4):
                            nc.tensor.matmul(
                                pw[:],
                                lhsT=win_sb[:, at * D + ec * 128:
                                            at * D + (ec + 1) * 128],
                                rhs=woutT_sb[:, at * D:(at + 1) * D],
                                start=(at == 0), stop=(at == 3))
                        nc.vector.tensor_copy(wcT[:, ec * D:(ec + 1) * D], pw[:])

            # residual inputs — late on the ACT queue, after the x stream,
            # so they don't steal DMA bandwidth from stepA's tail
            for b in range(B):
                nc.scalar.dma_start(
                    out=xres_sb[:, b * NSC * D:(b + 1) * NSC * D]
                        .rearrange("p (sc e) -> p sc e", sc=NSC),
                    in_=xres_d[b].rearrange("(sc p) e -> p sc e", p=128))
            if with_bias:
                for b in range(B):
                    for sc in range(NSC):
                        nc.vector.tensor_add(bsl(resb, b, sc),
                                             bsl(xres_sb, b, sc), bias_bc[:])
            res_src = resb if with_bias else xres_sb
            # tT copies for b=1: one on DVE, one on ACT (parallel)
            nc.vector.tensor_copy(
                tT[:, (1 * NE) * SLOC:(1 * NE + 2) * SLOC], pts[(1, 0)][:])
            nc.scalar.copy(
                tT[:, (1 * NE + 2) * SLOC:(1 * NE + 4) * SLOC], pts[(1, 1)][:])

            # ---- step B + fused epilogue + output DMA ----
            for b in range(B):
                for sc in range(NSC):
                    po = pa.tile([128, D], F32, name=f"po{b}_{sc}", tag="acc")
                    for et in range(NE):
                        nc.tensor.matmul(
                            po[:],
                            lhsT=tsl(tT, b, et)[:, sc * 128:(sc + 1) * 128],
                            rhs=wcT[:, et * D:(et + 1) * D],
                            start=(et == 0), stop=(et == 3))
                    nc.vector.scalar_tensor_tensor(
                        out=bsl(outbuf, b, sc),
                        in0=po[:],
                        scalar=rdT[:, sc:sc + 1],
                        in1=bsl(res_src, b, sc),
                        op0=MUL, op1=ADD)
                    dma_eng = nc.sync if b == 0 else nc.scalar
                    dma_eng.dma_start(
                        out=out_d[b, sc * 128:(sc + 1) * 128, :],
                        in_=bsl(outbuf, b, sc))

    nc.compile()
    return nc


def _get_nc(with_bias=True):
    key = ("nc", with_bias)
    if key not in _CACHE:
        _CACHE[key] = _build_nc(with_bias)
    return _CACHE[key]


def prep_in_maps(x, routes, distances, W_in, W_out, b_out):
    """Host-side sharding/marshalling: per-core input dicts."""
    import ml_dtypes
    import concourse.mybir as mybir

    bf16 = ml_dtypes.bfloat16
    f8 = mybir.dt.np(mybir.dt.float8e4 if STEPA_FP8 else mybir.dt.bfloat16)
    x = np.ascontiguousarray(np.asarray(x, dtype=np.float32))
    routes = np.asarray(routes, dtype=np.int32)
    distances = np.ascontiguousarray(np.asarray(distances, dtype=np.float32))
    W_in_b = np.ascontiguousarray(np.asarray(W_in, dtype=np.float32)).astype(bf16)
    W_outT_b = np.ascontiguousarray(
        np.asarray(W_out, dtype=np.float32).T).astype(bf16)
    b_out = np.ascontiguousarray(np.asarray(b_out, dtype=np.float32)).reshape(1, D)

    x_8 = x.astype(f8)

    # Count matrix C^T[j, s] = multiplicity of j in routes[s, :], shipped as
    # ln(C) so the device computes C*exp(-d) = exp(lnC - d); C=0 -> -1e4
    # underflows exp to +0. Depends only on the int32 index tensor.
    flat = routes.astype(np.int64).ravel() * S + np.repeat(np.arange(S, dtype=np.int64), K)
    countsT = np.bincount(flat, minlength=S * S).reshape(S, S)
    with np.errstate(divide="ignore"):
        lnctT = np.log(countsT.astype(np.float32))
    lnctT[countsT == 0] = -1e4
    lnctT = lnctT.astype(bf16)
    distT = np.ascontiguousarray(distances.T).astype(bf16)

    in_maps = []
    for c in range(N_CORES):
        sl = slice(c * SLOC, (c + 1) * SLOC)
        in_maps.append({
            "x": x_8,
            "distT": np.ascontiguousarray(distT[:, sl]),
            "lnct": np.ascontiguousarray(lnctT[:, sl]),
            "xres": np.ascontiguousarray(x[:, sl, :]),
            "w_in": W_in_b,
            "w_outT": W_outT_b,
            "b_out": b_out,
        })
    return in_maps


def kernel(x, routes, distances, W_in, W_out, b_out):
    global LAST_RESULTS
    from concourse import bass_utils

    in_maps = prep_in_maps(x, routes, distances, W_in, W_out, b_out)
    with_bias = bool(np.any(np.asarray(b_out)))
    nc = _get_nc(with_bias)
    _CACHE["last_nc"] = nc
    res = bass_utils.run_bass_kernel_spmd(nc, in_maps, core_ids=list(range(N_CORES)))
    LAST_RESULTS = res
    out = np.concatenate([res.results[c]["out"] for c in range(N_CORES)], axis=1)
    return out


if __name__ == "__main__":
    rng = np.random.default_rng(0)
    inputs = {
        "x": rng.standard_normal((B, S, D), dtype=np.float32),
        "routes": rng.integers(0, S, (S, K)).astype(np.int32),
        "distances": rng.random((S, S), dtype=np.float32),
        "W_in": (rng.standard_normal((D, D), dtype=np.float32) / np.sqrt(D)).astype(np.float32),
        "W_out": (rng.standard_normal((D, D), dtype=np.float32) / np.sqrt(D)).astype(np.float32),
        "b_out": np.zeros(D, dtype=np.float32),
    }
    out = kernel(**inputs)
    print("out", out.shape, out.dtype)



# revision 7
# speedup vs baseline: 1.4462x; 1.4462x over previous
"""Trainium2 Bass kernel for CantorMultiheadFusion.

Reference math:
    h      = x @ W_in^T                        # [B,S,D]
    d[s,k] = distances[s, routes[s,k]]
    w      = softmax(-d, axis=-1)              # [S,K]
    fused  = sum_k w[s,k] * h[:, routes[s,k]]  # [B,S,D]  (head reshape is a no-op)
    out    = fused @ W_out^T + b_out + x

Because the fusion weights are shared across the feature dim, the gather
commutes with both projections:
    out = (A @ x) @ (W_out @ W_in)^T + b_out + x
where A[s,j] = C[s,j] * exp(-distances[s,j]) / denom(s),
      C[s,j] = #{k : routes[s,k] == j}   (integer multiplicity),
      denom(s) = sum_j C[s,j] * exp(-distances[s,j]).
Duplicated route entries share the same distance, so the count matrix C is
exact. On device this is computed as exp(ln(C) - D) with ln(C) built
host-side from the int32 routes tensor alone (index marshalling; C=0 maps
to -448 so exp underflows to +0). All float math on the actual inputs
(exp, normalization, matmuls, residual) runs on device.

Sharding: sequence-parallel over S across 8 cores (256 rows each). x is
replicated since the A@x contraction needs all S rows.

The whole matmul datapath runs in fp8e4m3 with DoubleRow perf mode
(2 contraction tiles per instruction, 0.5 PE cycles/row):
  stepA: t^T[e,s] += x[b,jpair](lhsT) @ numer[jpair](rhs)    (j contract)
  Wc   : Wc[e,i]  = sum_a (16*W_in)[a,e] * W_outT[a,i]       (a contract)
  stepB: out[s,i] += (16*t)^T[spair... e contract] @ Wc
fp8 range handling: W_in is pre-scaled by 16 on host (exact power of 2)
and t is scaled by 16 in the PSUM->SBUF copies, so Wc and tT sit in the
fp8e4m3 sweet spot. The combined 256x factor is folded into the softmax
denominator by using 256-valued ones in the denominator matmul (exact in
fp8), so the epilogue's rdT multiply removes it for free.

The numerator stream ships as ONE packed fp8 tensor per core
[S, lnct(256) | distT(256)] so every DMA row is 512B (full wire rate) and
each j-tile pair needs a single DMA/sub/exp. numer is fp8: the softmax
denominator is summed from the SAME fp8 values, so correlated quantization
error cancels in the normalized weights.

Residual ships as bf16 (separate from the fp8 x stream).
"""

import os
import sys

import numpy as np

for _p in ("/opt/trn_rl_repo",):
    if os.path.isdir(_p) and _p not in sys.path:
        sys.path.insert(0, _p)

# Some container snapshots lack antenv.axon_hooks (the axon NTFF profile
# hook); stub it so run_bass_kernel_spmd(trace=True) degrades gracefully.
def _ensure_axon_hooks_stub():
    import types
    try:
        import antenv.axon_hooks  # noqa: F401
    except ModuleNotFoundError:
        try:
            import antenv
        except ModuleNotFoundError:
            return
        _stub = types.ModuleType("antenv.axon_hooks")
        _stub.get_axon_ntff_profile_hook = lambda: None
        sys.modules["antenv.axon_hooks"] = _stub
        antenv.axon_hooks = _stub


_ensure_axon_hooks_stub()

B, S, D, K = 2, 2048, 512, 64
N_CORES = 8
SLOC = S // N_CORES          # 256 sequence rows per core
NJ = S // 128                # 16 contraction tiles
NP = NJ // 2                 # 8 DoubleRow contraction pairs
NE = D // 128                # 4 feature chunks
NSC = SLOC // 128            # 2 seq chunks per core

WSCALE = 16.0                # host scale on W_in (exact power of 2)
# t is PRE-normalization (sigma ~5-10, denominator ~40 applied later), so
# it already sits in fp8 range unscaled; scaling by 16 overflows +-448.
TSCALE = 1.0
ONES_VAL = WSCALE * TSCALE   # folded into the softmax denominator

# ld groups (pair ranges) pace the numerator critical path; x groups are
# coarser since x is only needed by stepA itself.
LD_GROUPS = [(0, 1), (1, 1), (2, 2), (4, 2), (6, 2)]
X_GROUPS = {0: (0, 2), 2: (2, 2), 4: (4, 2), 6: (6, 2)}

_CACHE = {}
LAST_RESULTS = None


def _build_nc(with_bias=True):
    import concourse.bacc as bacc
    import concourse.mybir as mybir
    import concourse.tile as tile

    F32 = mybir.dt.float32
    BF16 = mybir.dt.bfloat16
    F8 = mybir.dt.float8e4
    MUL = mybir.AluOpType.mult
    ADD = mybir.AluOpType.add
    DR = mybir.MatmulPerfMode.DoubleRow
    EXP = mybir.ActivationFunctionType.Exp

    nc = bacc.Bacc("TRN2", target_bir_lowering=False, debug=False, num_devices=1)

    x_d = nc.dram_tensor("x", [B, S, D], F8, kind="ExternalInput").ap()
    ld_d = nc.dram_tensor("ldpack", [S, 2 * SLOC], F8, kind="ExternalInput").ap()
    xres_d = nc.dram_tensor("xres", [B, SLOC, D], BF16, kind="ExternalInput").ap()
    wpack_d = nc.dram_tensor("wpack", [2, D, D], F8, kind="ExternalInput").ap()
    bout_d = nc.dram_tensor("b_out", [1, D], F32, kind="ExternalInput").ap()
    out_d = nc.dram_tensor("out", [B, SLOC, D], F32, kind="ExternalOutput").ap()

    with tile.TileContext(nc) as tc:
        with (
            tc.tile_pool(name="big", bufs=1) as big,
            tc.tile_pool(name="ldstream", bufs=2) as ldstream,
            tc.tile_pool(name="sstream", bufs=2) as sstream,
            tc.tile_pool(name="pa", bufs=4, space="PSUM") as pa,
            tc.tile_pool(name="pden", bufs=1, space="PSUM") as pdenp,
            tc.tile_pool(name="ptr", bufs=2, space="PSUM") as ptr,
        ):
            # ---- persistent SBUF ----
            xbuf = big.tile([128, B * NJ, D], F8)       # [128j, (b,j), 512e]
            numer = big.tile([128, NJ, SLOC], F8)       # [128j, j, 256s]
            w_sb = big.tile([128, 8, D], F8)            # [128a, (w,t), *]
            wcT = big.tile([128, NE, D], F8)            # [128e, ec, 512i]
            ones2 = big.tile([128, 2], F8)
            xres_sb = big.tile([128, B * NSC, D], BF16)
            tT = big.tile([128, B * NE, SLOC], F8)      # [128e, (b,ec), 256s]
            outbuf = big.tile([128, B * NSC, D], F32)
            rdT = big.tile([128, NSC], F32)
            if with_bias:
                bias_sb = big.tile([1, D], F32)
                bias_bc = big.tile([128, D], F32)
                ones_r = big.tile([1, 128], F32)
                resb = big.tile([128, B * NSC, D], F32)

            nc.vector.memset(ones2[:], ONES_VAL)

            # stepA PSUM: bank (b,p) holds ec=2p (cols :SLOC) and ec=2p+1
            # (cols SLOC:). Exactly one start=True per bank.
            pdT = pdenp.tile([128, NSC], F32)
            pts = {(b, p): pa.tile([128, 2 * SLOC], F32,
                                   name=f"pts{b}_{p}", tag="acc")
                   for b in range(B) for p in range(NE // 2)}

            def pta(b, ec):
                return pts[(b, ec // 2)][:, (ec % 2) * SLOC:(ec % 2 + 1) * SLOC]

            if with_bias:
                nc.sync.dma_start(out=bias_sb[:1, :], in_=bout_d[:, :])
                nc.vector.memset(ones_r[:], 1.0)
                pb = ptr.tile([128, D], F32, name="pb", tag="tr")
                nc.tensor.matmul(pb[:], lhsT=ones_r[:1, :], rhs=bias_sb[:1, :],
                                 start=True, stop=True)
                nc.vector.tensor_copy(bias_bc[:], pb[:])

            # weights early on the ACT queue; xres early on the Pool/SWDGE
            # queue (bypasses HWDGE entirely)
            nc.scalar.dma_start(
                out=w_sb[:],
                in_=wpack_d.rearrange("w (t p) e -> p (w t) e", p=128))
            nc.gpsimd.dma_start(
                out=xres_sb[:],
                in_=xres_d.rearrange("b (sc p) e -> p (b sc) e", p=128))

            def npair(pp):
                return numer[:, 2 * pp:2 * pp + 2, :]

            def denom_mm(pp):
                n3 = npair(pp)
                for sc in range(NSC):
                    nc.tensor.matmul(
                        pdT[:, sc:sc + 1],
                        lhsT=n3[:, :, sc * 128:(sc + 1) * 128],
                        rhs=ones2[:].rearrange("p (j o) -> p j o", j=2),
                        start=(pp == 0 and sc == 0),
                        stop=(pp == NP - 1 and sc == NSC - 1),
                        perf_mode=DR, skip_group_check=True)

            def stepa(pp, b):
                x3 = xbuf[:, b * NJ + 2 * pp:b * NJ + 2 * pp + 2, :]
                n3 = npair(pp)
                for ec in range(NE):
                    nc.tensor.matmul(
                        pta(b, ec),
                        lhsT=x3[:, :, ec * 128:(ec + 1) * 128],
                        rhs=n3,
                        start=(pp == 0 and ec % 2 == 0),
                        stop=(pp == NP - 1 and ec % 2 == 1),
                        perf_mode=DR, skip_group_check=True)

            def tt_copies(b, split):
                # copy PSUM->SBUF fp8 (scale by TSCALE if != 1)
                for p in range(NE // 2):
                    dst = tT[:, b * NE + 2 * p:b * NE + 2 * p + 2, :]
                    if split and p == 1:
                        if TSCALE != 1.0:
                            nc.scalar.activation(
                                dst, pts[(b, p)][:],
                                mybir.ActivationFunctionType.Copy, scale=TSCALE)
                        else:
                            nc.scalar.copy(dst, pts[(b, p)][:])
                    elif TSCALE != 1.0:
                        nc.vector.tensor_scalar_mul(dst, pts[(b, p)][:], TSCALE)
                    else:
                        nc.vector.tensor_copy(dst, pts[(b, p)][:])

            LASTG = len(LD_GROUPS) - 1
            for g, (p0, pn) in enumerate(LD_GROUPS):
                jn = 2 * pn
                j0 = 2 * p0
                ld_t = ldstream.tile([128, jn, 2 * SLOC], F8,
                                     name=f"ld{g}", tag="ld")
                nc.sync.dma_start(
                    out=ld_t[:],
                    in_=ld_d[j0 * 128:(j0 + jn) * 128, :]
                        .rearrange("(j p) c -> p j c", p=128))
                if p0 in X_GROUPS:
                    xp0, xpn = X_GROUPS[p0]
                    for b in range(B):
                        nc.sync.dma_start(
                            out=xbuf[:, b * NJ + 2 * xp0:
                                     b * NJ + 2 * (xp0 + xpn), :],
                            in_=x_d[b, 2 * xp0 * 128:2 * (xp0 + xpn) * 128, :]
                                .rearrange("(j p) e -> p j e", p=128))
                # numerator: per pair sub (DVE) + exp (ACT)
                for q in range(pn):
                    pp = p0 + q
                    sb_t = sstream.tile([128, 2, SLOC], BF16,
                                        name=f"sb{g}_{q}", tag="sb")
                    nc.vector.tensor_sub(sb_t[:],
                                         ld_t[:, 2 * q:2 * q + 2, :SLOC],
                                         ld_t[:, 2 * q:2 * q + 2, SLOC:])
                    nc.scalar.activation(npair(pp), sb_t[:], EXP)
                # stepA for this group's pairs
                if g < LASTG:
                    for q in range(pn):
                        pp = p0 + q
                        denom_mm(pp)
                        stepa(pp, 0)
                        stepa(pp, 1)
                else:
                    for q in range(pn):
                        pp = p0 + q
                        denom_mm(pp)
                        stepa(pp, 0)
                    tt_copies(0, split=False)
                    nc.vector.reciprocal(rdT[:], pdT[:])
                    for q in range(pn):
                        stepa(p0 + q, 1)
                    tt_copies(1, split=True)
                if g == 0:
                    # Wc[e,i] = sum_a (16*W_in)[a,e] * W_outT[a,i], fp8
                    # DoubleRow, emitted after the first stepA group so PE
                    # starts on stream data ASAP
                    for ec in range(NE):
                        pw = ptr.tile([128, D], F32, name=f"pw{ec}", tag="tr")
                        for ap_ in range(2):
                            nc.tensor.matmul(
                                pw[:],
                                lhsT=w_sb[:, 2 * ap_:2 * ap_ + 2,
                                          ec * 128:(ec + 1) * 128],
                                rhs=w_sb[:, 4 + 2 * ap_:4 + 2 * ap_ + 2, :],
                                start=(ap_ == 0), stop=(ap_ == 1),
                                perf_mode=DR)
                        if ec < 2:
                            nc.vector.tensor_copy(wcT[:, ec, :], pw[:])
                        else:
                            nc.scalar.copy(wcT[:, ec, :], pw[:])

            if with_bias:
                for b in range(B):
                    for sc in range(NSC):
                        nc.vector.tensor_add(resb[:, b * NSC + sc, :],
                                             xres_sb[:, b * NSC + sc, :],
                                             bias_bc[:])
            res_src = resb if with_bias else xres_sb

            # ---- step B + fused epilogue + output DMA ----
            for b in range(B):
                t3 = tT[:, b * NE:(b + 1) * NE, :]
                for sc in range(NSC):
                    po = pa.tile([128, D], F32, name=f"po{b}_{sc}", tag="acc")
                    for ep in range(2):
                        nc.tensor.matmul(
                            po[:],
                            lhsT=t3[:, 2 * ep:2 * ep + 2,
                                    sc * 128:(sc + 1) * 128],
                            rhs=wcT[:, 2 * ep:2 * ep + 2, :],
                            start=(ep == 0), stop=(ep == 1),
                            perf_mode=DR)
                    nc.vector.scalar_tensor_tensor(
                        out=outbuf[:, b * NSC + sc, :],
                        in0=po[:],
                        scalar=rdT[:, sc:sc + 1],
                        in1=res_src[:, b * NSC + sc, :],
                        op0=MUL, op1=ADD)
                    dma_eng = nc.sync if b == 0 else nc.scalar
                    dma_eng.dma_start(
                        out=out_d[b, sc * 128:(sc + 1) * 128, :],
                        in_=outbuf[:, b * NSC + sc, :])

    nc.compile()
    return nc


def _get_nc(with_bias=True):
    key = ("nc", with_bias)
    if key not in _CACHE:
        _CACHE[key] = _build_nc(with_bias)
    return _CACHE[key]


def prep_in_maps(x, routes, distances, W_in, W_out, b_out):
    """Host-side sharding/marshalling: per-core input dicts."""
    import ml_dtypes
    import concourse.mybir as mybir

    bf16 = ml_dtypes.bfloat16
    f8 = mybir.dt.np(mybir.dt.float8e4)
    x = np.ascontiguousarray(np.asarray(x, dtype=np.float32))
    routes = np.asarray(routes, dtype=np.int32)
    distances = np.ascontiguousarray(np.asarray(distances, dtype=np.float32))
    b_out = np.ascontiguousarray(np.asarray(b_out, dtype=np.float32)).reshape(1, D)

    wpack = np.empty((2, D, D), dtype=np.float32)
    wpack[0] = np.asarray(W_in, dtype=np.float32) * WSCALE
    wpack[1] = np.asarray(W_out, dtype=np.float32).T
    wpack = wpack.astype(f8)

    x_8 = x.astype(f8)
    xres_b = x.astype(bf16)

    # Count matrix C^T[j, s] = multiplicity of j in routes[s, :], shipped as
    # ln(C) so the device computes C*exp(-d) = exp(lnC - d); C=0 -> -448
    # underflows exp to +0. Depends only on the int32 index tensor.
    flat = routes.astype(np.int64).ravel() * S + np.repeat(np.arange(S, dtype=np.int64), K)
    countsT = np.bincount(flat, minlength=S * S).reshape(S, S)
    with np.errstate(divide="ignore"):
        lnctT = np.log(countsT.astype(np.float32))
    lnctT[countsT == 0] = -448.0
    distT = distances.T

    in_maps = []
    for c in range(N_CORES):
        sl = slice(c * SLOC, (c + 1) * SLOC)
        ldpack = np.empty((S, 2 * SLOC), dtype=np.float32)
        ldpack[:, :SLOC] = lnctT[:, sl]
        ldpack[:, SLOC:] = distT[:, sl]
        in_maps.append({
            "x": x_8,
            "ldpack": ldpack.astype(f8),
            "xres": np.ascontiguousarray(xres_b[:, sl, :]),
            "wpack": wpack,
            "b_out": b_out,
        })
    return in_maps


def kernel(x, routes, distances, W_in, W_out, b_out):
    global LAST_RESULTS
    from concourse import bass_utils

    in_maps = prep_in_maps(x, routes, distances, W_in, W_out, b_out)
    with_bias = bool(np.any(np.asarray(b_out)))
    nc = _get_nc(with_bias)
    _CACHE["last_nc"] = nc
    res = bass_utils.run_bass_kernel_spmd(nc, in_maps, core_ids=list(range(N_CORES)))
    LAST_RESULTS = res
    out = np.concatenate([res.results[c]["out"] for c in range(N_CORES)], axis=1)
    return out


if __name__ == "__main__":
    rng = np.random.default_rng(0)
    inputs = {
        "x": rng.standard_normal((B, S, D), dtype=np.float32),
        "routes": rng.integers(0, S, (S, K)).astype(np.int32),
        "distances": rng.random((S, S), dtype=np.float32),
        "W_in": (rng.standard_normal((D, D), dtype=np.float32) / np.sqrt(D)).astype(np.float32),
        "W_out": (rng.standard_normal((D, D), dtype=np.float32) / np.sqrt(D)).astype(np.float32),
        "b_out": np.zeros(D, dtype=np.float32),
    }
    out = kernel(**inputs)
    print("out", out.shape, out.dtype)


# revision 10
# speedup vs baseline: 1.6607x; 1.1483x over previous
"""Trainium2 Bass kernel for CantorMultiheadFusion.

Reference math:
    h      = x @ W_in^T                        # [B,S,D]
    d[s,k] = distances[s, routes[s,k]]
    w      = softmax(-d, axis=-1)              # [S,K]
    fused  = sum_k w[s,k] * h[:, routes[s,k]]  # [B,S,D]  (head reshape is a no-op)
    out    = fused @ W_out^T + b_out + x

Because the fusion weights are shared across the feature dim, the gather
commutes with both projections:
    out = (A @ x) @ (W_out @ W_in)^T + b_out + x
where A[s,j] = C[s,j] * exp(-distances[s,j]) / denom(s),
      C[s,j] = #{k : routes[s,k] == j}   (integer multiplicity),
      denom(s) = sum_j C[s,j] * exp(-distances[s,j]).
Duplicated route entries share the same distance, so the count matrix C is
exact. On device this is computed as exp(ln(C) - D) with ln(C) built
host-side from the int32 routes tensor alone (index marshalling; C=0 maps
to -448 so exp underflows to +0). All float math on the actual inputs
(exp, normalization, matmuls, residual) runs on device.

Sharding: sequence-parallel over S across 8 cores (256 rows each). x is
replicated since the A@x contraction needs all S rows.

The matmul datapath runs in fp8e4m3 with DoubleRow perf mode (two
contraction tiles per instruction, 0.5 PE cycles/row):
  stepA: t^T[e,s] += x[b,jpair](lhsT) @ numer[jpair](rhs)    (j contract)
  Wc   : Wc[e,i]  = sum_a (16*W_in)[a,e] * W_outT[a,i]       (a contract)
  stepB: out[s,i] += tT[epair](lhsT) @ Wc[epair](rhs)        (e contract)
fp8 range handling: W_in is pre-scaled by 16 on host (exact power of 2) so
Wc sits in the fp8e4m3 sweet spot; the factor is folded into the softmax
denominator by using 16-valued ones in the denominator matmul (exact in
fp8), so the epilogue's rdT multiply removes it for free. t is
pre-normalization (sigma ~5-10) and ships through fp8 unscaled.

The numerator stream ships as ONE packed fp8 tensor per core
[S, lnct(256) | distT(256)] so every DMA row is 512B (full wire rate) and
each j-tile pair needs a single DMA/sub/exp. numer is fp8: the softmax
denominator is summed from the SAME fp8 values, so correlated quantization
error cancels in the normalized weights.

Schedule: the kernel is wire-bound (DMA bytes/360GB/s ~ 11.6us of input),
so the batches are software-pipelined: the full b0 pass (stream, stepA,
tT copies, stepB, epilogue, output) runs while b1's x stream is still on
the wire, hiding half the epilogue latency. All input DMAs ride one queue
(SP) in explicit wire order: ld0|xb0a|ld1|ld2|ld3|xb0b|W|xres|xb1a|xb1b.
Output is written bf16 and upcast to f32 on the host (pure dtype
marshalling) to halve output wire time.
"""

import os
import sys

import numpy as np

for _p in ("/opt/trn_rl_repo",):
    if os.path.isdir(_p) and _p not in sys.path:
        sys.path.insert(0, _p)

# Some container snapshots lack antenv.axon_hooks (the axon NTFF profile
# hook); stub it so run_bass_kernel_spmd(trace=True) degrades gracefully.
def _ensure_axon_hooks_stub():
    import types
    try:
        import antenv.axon_hooks  # noqa: F401
    except ModuleNotFoundError:
        try:
            import antenv
        except ModuleNotFoundError:
            return
        _stub = types.ModuleType("antenv.axon_hooks")
        _stub.get_axon_ntff_profile_hook = lambda: None
        sys.modules["antenv.axon_hooks"] = _stub
        antenv.axon_hooks = _stub


_ensure_axon_hooks_stub()

B, S, D, K = 2, 2048, 512, 64
N_CORES = 8
SLOC = S // N_CORES          # 256 sequence rows per core
NJ = S // 128                # 16 contraction tiles
NP = NJ // 2                 # 8 DoubleRow contraction pairs
NE = D // 128                # 4 feature chunks
NSC = SLOC // 128            # 2 seq chunks per core

WSCALE = 16.0                # host scale on W_in (exact power of 2)
ONES_VAL = WSCALE           # folded into the softmax denominator

LD_GROUPS = [(0, 2), (2, 2), (4, 2), (6, 2)]   # numerator stream (pairs)
X_GROUPS = [(0, 4), (4, 4)]                    # x stream per batch (pairs)

_CACHE = {}
LAST_RESULTS = None


def _build_nc(with_bias=True):
    import concourse.bacc as bacc
    import concourse.mybir as mybir
    import concourse.tile as tile

    F32 = mybir.dt.float32
    BF16 = mybir.dt.bfloat16
    F8 = mybir.dt.float8e4
    MUL = mybir.AluOpType.mult
    ADD = mybir.AluOpType.add
    DR = mybir.MatmulPerfMode.DoubleRow
    EXP = mybir.ActivationFunctionType.Exp

    nc = bacc.Bacc("TRN2", target_bir_lowering=False, debug=False, num_devices=1)

    x_d = nc.dram_tensor("x", [B, S, D], F8, kind="ExternalInput").ap()
    ld_d = nc.dram_tensor("ldpack", [S, 2 * SLOC], F8, kind="ExternalInput").ap()
    xres_d = nc.dram_tensor("xres", [B, SLOC, D], BF16, kind="ExternalInput").ap()
    wpack_d = nc.dram_tensor("wpack", [2, D, D], F8, kind="ExternalInput").ap()
    bout_d = nc.dram_tensor("b_out", [1, D], F32, kind="ExternalInput").ap()
    out_d = nc.dram_tensor("out", [B, SLOC, D], BF16, kind="ExternalOutput").ap()

    with tile.TileContext(nc) as tc:
        with (
            tc.tile_pool(name="big", bufs=1) as big,
            tc.tile_pool(name="ldstream", bufs=4) as ldstream,
            tc.tile_pool(name="sstream", bufs=2) as sstream,
            tc.tile_pool(name="pa", bufs=4, space="PSUM") as pa,
            tc.tile_pool(name="pden", bufs=1, space="PSUM") as pdenp,
            tc.tile_pool(name="ptr", bufs=2, space="PSUM") as ptr,
        ):
            # ---- persistent SBUF ----
            xb = [big.tile([128, NJ, D], F8, name=f"xb{b}") for b in range(B)]
            numer = big.tile([128, NJ, SLOC], F8)       # [128j, j, 256s]
            w_sb = big.tile([128, 8, D], F8)            # [128a, (w,t), *]
            wcT = big.tile([128, NE, D], F8)            # [128e, ec, 512i]
            ones2 = big.tile([128, 2], F8)
            xres_sb = big.tile([128, B * NSC, D], BF16)
            tT = big.tile([128, B * NE, SLOC], F8)      # [128e, (b,ec), 256s]
            outbuf = big.tile([128, B * NSC, D], BF16)
            rdT = big.tile([128, NSC], F32)
            if with_bias:
                bias_sb = big.tile([1, D], F32)
                bias_bc = big.tile([128, D], F32)
                ones_r = big.tile([1, 128], F32)
                resb = big.tile([128, B * NSC, D], F32)

            nc.vector.memset(ones2[:], ONES_VAL)

            # stepA PSUM: bank (b,p) holds ec=2p (cols :SLOC) and ec=2p+1
            # (cols SLOC:). Exactly one start=True per bank.
            pdT = pdenp.tile([128, NSC], F32)
            pts = {(b, p): pa.tile([128, 2 * SLOC], F32,
                                   name=f"pts{b}_{p}", tag="acc")
                   for b in range(B) for p in range(NE // 2)}

            def pta(b, ec):
                return pts[(b, ec // 2)][:, (ec % 2) * SLOC:(ec % 2 + 1) * SLOC]

            if with_bias:
                nc.scalar.dma_start(out=bias_sb[:1, :], in_=bout_d[:, :])
                nc.vector.memset(ones_r[:], 1.0)
                pb = ptr.tile([128, D], F32, name="pb", tag="tr")
                nc.tensor.matmul(pb[:], lhsT=ones_r[:1, :], rhs=bias_sb[:1, :],
                                 start=True, stop=True)
                nc.vector.tensor_copy(bias_bc[:], pb[:])

            def npair(pp):
                return numer[:, 2 * pp:2 * pp + 2, :]

            def denom_mm(pp):
                n3 = npair(pp)
                for sc in range(NSC):
                    nc.tensor.matmul(
                        pdT[:, sc:sc + 1],
                        lhsT=n3[:, :, sc * 128:(sc + 1) * 128],
                        rhs=ones2[:].rearrange("p (j o) -> p j o", j=2),
                        start=(pp == 0 and sc == 0),
                        stop=(pp == NP - 1 and sc == NSC - 1),
                        perf_mode=DR, skip_group_check=True)

            def stepa(pp, b):
                x3 = xb[b][:, 2 * pp:2 * pp + 2, :]
                n3 = npair(pp)
                for ec in range(NE):
                    nc.tensor.matmul(
                        pta(b, ec),
                        lhsT=x3[:, :, ec * 128:(ec + 1) * 128],
                        rhs=n3,
                        start=(pp == 0 and ec % 2 == 0),
                        stop=(pp == NP - 1 and ec % 2 == 1),
                        perf_mode=DR, skip_group_check=True)

            def tt_copies(b):
                # PSUM->SBUF fp8, one bank on DVE and one on ACT in parallel
                nc.vector.tensor_copy(tT[:, b * NE:b * NE + 2, :],
                                      pts[(b, 0)][:])
                nc.scalar.copy(tT[:, b * NE + 2:b * NE + 4, :],
                               pts[(b, 1)][:])

            def x_dma(b, xg):
                xp0, xpn = X_GROUPS[xg]
                nc.sync.dma_start(
                    out=xb[b][:, 2 * xp0:2 * (xp0 + xpn), :],
                    in_=x_d[b, 2 * xp0 * 128:2 * (xp0 + xpn) * 128, :]
                        .rearrange("(j p) e -> p j e", p=128))

            def stepb_epilogue(b, sc, out_eng):
                po = pa.tile([128, D], F32, name=f"po{b}_{sc}", tag="acc")
                t3 = tT[:, b * NE:(b + 1) * NE, :]
                for ep in range(2):
                    nc.tensor.matmul(
                        po[:],
                        lhsT=t3[:, 2 * ep:2 * ep + 2, sc * 128:(sc + 1) * 128],
                        rhs=wcT[:, 2 * ep:2 * ep + 2, :],
                        start=(ep == 0), stop=(ep == 1),
                        perf_mode=DR)
                res = resb if with_bias else xres_sb
                nc.vector.scalar_tensor_tensor(
                    out=outbuf[:, b * NSC + sc, :],
                    in0=po[:],
                    scalar=rdT[:, sc:sc + 1],
                    in1=res[:, b * NSC + sc, :],
                    op0=MUL, op1=ADD)
                out_eng.dma_start(
                    out=out_d[b, sc * 128:(sc + 1) * 128, :],
                    in_=outbuf[:, b * NSC + sc, :])

            # ---- all input DMAs upfront on SP, in explicit wire order:
            # ld0 | xb0a | ld1 ld2 ld3 | xb0b | W | xres | xb1a xb1b ----
            ld_ts = []
            for g, (p0, pn) in enumerate(LD_GROUPS):
                jn, j0 = 2 * pn, 2 * p0
                ld_t = ldstream.tile([128, jn, 2 * SLOC], F8,
                                     name=f"ld{g}", tag="ld")
                ld_ts.append(ld_t)
                nc.sync.dma_start(
                    out=ld_t[:],
                    in_=ld_d[j0 * 128:(j0 + jn) * 128, :]
                        .rearrange("(j p) c -> p j c", p=128))
                if g == 0:
                    x_dma(0, 0)
            x_dma(0, 1)
            nc.sync.dma_start(
                out=w_sb[:],
                in_=wpack_d.rearrange("w (t p) e -> p (w t) e", p=128))
            nc.sync.dma_start(
                out=xres_sb[:],
                in_=xres_d.rearrange("b (sc p) e -> p (b sc) e", p=128))
            x_dma(1, 0)
            x_dma(1, 1)

            # ---- numerator + b0 stepA pass ----
            for g, (p0, pn) in enumerate(LD_GROUPS):
                ld_t = ld_ts[g]
                for q in range(pn):
                    pp = p0 + q
                    sb_t = sstream.tile([128, 2, SLOC], BF16,
                                        name=f"sb{g}_{q}", tag="sb")
                    nc.vector.tensor_sub(sb_t[:],
                                         ld_t[:, 2 * q:2 * q + 2, :SLOC],
                                         ld_t[:, 2 * q:2 * q + 2, SLOC:])
                    nc.scalar.activation(npair(pp), sb_t[:], EXP)
                    denom_mm(pp)
                    stepa(pp, 0)

            tt_copies(0)
            nc.vector.reciprocal(rdT[:], pdT[:])

            # Wc[e,i] = sum_a (16*W_in)[a,e] * W_outT[a,i], fp8 DoubleRow
            for ec in range(NE):
                pw = ptr.tile([128, D], F32, name=f"pw{ec}", tag="tr")
                for ap_ in range(2):
                    nc.tensor.matmul(
                        pw[:],
                        lhsT=w_sb[:, 2 * ap_:2 * ap_ + 2,
                                  ec * 128:(ec + 1) * 128],
                        rhs=w_sb[:, 4 + 2 * ap_:4 + 2 * ap_ + 2, :],
                        start=(ap_ == 0), stop=(ap_ == 1),
                        perf_mode=DR)
                if ec < 2:
                    nc.vector.tensor_copy(wcT[:, ec, :], pw[:])
                else:
                    nc.scalar.copy(wcT[:, ec, :], pw[:])

            if with_bias:
                for b in range(B):
                    for sc in range(NSC):
                        nc.vector.tensor_add(resb[:, b * NSC + sc, :],
                                             xres_sb[:, b * NSC + sc, :],
                                             bias_bc[:])

            # ---- b0 stepB/epilogue overlapped with b1 stepA ----
            stepb_epilogue(0, 0, nc.sync)
            stepb_epilogue(0, 1, nc.sync)
            for pp in range(NP):
                stepa(pp, 1)
            tt_copies(1)
            stepb_epilogue(1, 0, nc.scalar)
            stepb_epilogue(1, 1, nc.sync)

    nc.compile()
    return nc


def _get_nc(with_bias=True):
    key = ("nc", with_bias)
    if key not in _CACHE:
        _CACHE[key] = _build_nc(with_bias)
    return _CACHE[key]


def prep_in_maps(x, routes, distances, W_in, W_out, b_out):
    """Host-side sharding/marshalling: per-core input dicts."""
    import ml_dtypes
    import concourse.mybir as mybir

    bf16 = ml_dtypes.bfloat16
    f8 = mybir.dt.np(mybir.dt.float8e4)
    x = np.ascontiguousarray(np.asarray(x, dtype=np.float32))
    routes = np.asarray(routes, dtype=np.int32)
    distances = np.ascontiguousarray(np.asarray(distances, dtype=np.float32))
    b_out = np.ascontiguousarray(np.asarray(b_out, dtype=np.float32)).reshape(1, D)

    wpack = np.empty((2, D, D), dtype=np.float32)
    wpack[0] = np.asarray(W_in, dtype=np.float32) * WSCALE
    wpack[1] = np.asarray(W_out, dtype=np.float32).T
    wpack = wpack.astype(f8)

    x_8 = x.astype(f8)
    xres_b = x.astype(bf16)

    # Count matrix C^T[j, s] = multiplicity of j in routes[s, :], shipped as
    # ln(C) so the device computes C*exp(-d) = exp(lnC - d); C=0 -> -448
    # underflows exp to +0. Depends only on the int32 index tensor.
    flat = routes.astype(np.int64).ravel() * S + np.repeat(np.arange(S, dtype=np.int64), K)
    countsT = np.bincount(flat, minlength=S * S).reshape(S, S)
    with np.errstate(divide="ignore"):
        lnctT = np.log(countsT.astype(np.float32))
    lnctT[countsT == 0] = -448.0
    distT = distances.T

    in_maps = []
    for c in range(N_CORES):
        sl = slice(c * SLOC, (c + 1) * SLOC)
        ldpack = np.empty((S, 2 * SLOC), dtype=np.float32)
        ldpack[:, :SLOC] = lnctT[:, sl]
        ldpack[:, SLOC:] = distT[:, sl]
        in_maps.append({
            "x": x_8,
            "ldpack": ldpack.astype(f8),
            "xres": np.ascontiguousarray(xres_b[:, sl, :]),
            "wpack": wpack,
            "b_out": b_out,
        })
    return in_maps


def kernel(x, routes, distances, W_in, W_out, b_out):
    global LAST_RESULTS
    from concourse import bass_utils

    in_maps = prep_in_maps(x, routes, distances, W_in, W_out, b_out)
    with_bias = bool(np.any(np.asarray(b_out)))
    nc = _get_nc(with_bias)
    _CACHE["last_nc"] = nc
    res = bass_utils.run_bass_kernel_spmd(nc, in_maps, core_ids=list(range(N_CORES)))
    LAST_RESULTS = res
    out = np.concatenate(
        [res.results[c]["out"].astype(np.float32) for c in range(N_CORES)],
        axis=1)
    return out


if __name__ == "__main__":
    rng = np.random.default_rng(0)
    inputs = {
        "x": rng.standard_normal((B, S, D), dtype=np.float32),
        "routes": rng.integers(0, S, (S, K)).astype(np.int32),
        "distances": rng.random((S, S), dtype=np.float32),
        "W_in": (rng.standard_normal((D, D), dtype=np.float32) / np.sqrt(D)).astype(np.float32),
        "W_out": (rng.standard_normal((D, D), dtype=np.float32) / np.sqrt(D)).astype(np.float32),
        "b_out": np.zeros(D, dtype=np.float32),
    }
    out = kernel(**inputs)
    print("out", out.shape, out.dtype)
